# revision 1
# baseline (speedup 1.0000x reference)
"""Trainium2 Bass kernel for nn_DAInsHead (moe_routing).

Per-row hard-routed 3-layer MLP: rows with levels[i]==l get
    out[i] = W3[l].T @ relu(W2[l].T @ relu(W1[l].T @ x[i] + b1[l]) + b2[l]) + b3[l]

Strategy (vs the reference's dense 4x-redundant masked compute):
  * Host: stable-sort rows by level, deal each level's rows evenly to the 8
    cores, pad each (core, level) segment to a shared per-level capacity
    (multiple of 256), and transpose to feature-major xT [D, R_core] so the
    device needs no on-chip transposes.
  * Device (identical SPMD program on 8 cores): for each level, keep that
    level's W1/W2 resident in SBUF (f32r) and stream row tiles of 512:
    L1/L2 are K=8-chunk accumulated 128x128x512 f32r matmuls (full PE rate,
    ~1 cycle/row) with DVE relu+bias eviction; L3 is a K-chunked matvec.
  * Host: scatter per-core outputs back to original row order.

f32r (fp32 data issued to the PE in reduced-precision streaming mode) gives
~1.5e-4 scale-relative error per 1024-deep matmul vs 2e-3 for bf16, at 4x
the throughput of plain fp32 matmul.
"""
import os
import sys

sys.path.insert(0, "/opt/trn_rl_repo")

import numpy as np

import concourse.bacc as bacc
import concourse.mybir as mybir
import concourse.tile as tile
from concourse.bass_utils import run_bass_kernel_spmd

F32 = mybir.dt.float32
F32R = mybir.dt.float32r
ADD = mybir.AluOpType.add
MAX = mybir.AluOpType.max

NC = 8          # cores
L = 4           # levels
D = 1024        # in features
H = 1024        # hidden
KC = D // 128   # contraction chunks

LAST_RESULTS = None       # BassKernelResults of the most recent run (for test.py)
_PROGRAM_CACHE = {}


def _row_tiles(c):
    """Split a per-level capacity (multiple of 256) into row-tile sizes."""
    tiles = [512] * (c // 512)
    if c % 512:
        tiles.append(c % 512)
    return tiles


def _build_program(caps):
    """Build + compile the SPMD program for per-level capacities `caps`."""
    r_core = sum(caps)
    nc = bacc.Bacc("TRN2", target_bir_lowering=False, debug=False, num_devices=NC)
    xT = nc.dram_tensor("xT", [D, r_core], F32R, kind="ExternalInput")
    W1 = nc.dram_tensor("W1", [L, D, H], F32R, kind="ExternalInput")
    W2 = nc.dram_tensor("W2", [L, H, H], F32R, kind="ExternalInput")
    W3 = nc.dram_tensor("W3", [L, H, 1], F32R, kind="ExternalInput")
    b1 = nc.dram_tensor("b1", [L, H], F32, kind="ExternalInput")
    b2 = nc.dram_tensor("b2", [L, H], F32, kind="ExternalInput")
    b3 = nc.dram_tensor("b3", [L, 1], F32, kind="ExternalInput")
    out = nc.dram_tensor("out", [1, r_core], F32, kind="ExternalOutput")

    xT_r = xT.rearrange("(kc p) r -> p kc r", p=128)  # [128, KC, r_core]

    with tile.TileContext(nc) as tc:
        with (
            tc.tile_pool(name="wpool", bufs=2) as wpool,
            tc.tile_pool(name="bpool", bufs=2) as bpool,
            tc.tile_pool(name="xpool", bufs=2) as xpool,
            tc.tile_pool(name="hpool", bufs=1) as hpool,
            tc.tile_pool(name="opool", bufs=3) as opool,
            tc.tile_pool(name="ps", bufs=6, space="PSUM") as ps,
            tc.tile_pool(name="ps3", bufs=2, space="PSUM") as ps3,
        ):
            off = 0
            for lvl in range(L):
                cap = caps[lvl]
                if cap == 0:
                    continue
                # Per-kc weight tiles so the first matmuls only wait on the
                # first 512KB of weight DMA, and level l+1 prefetch
                # double-buffers against level l (bufs=2 per tag).
                w1k = []
                w2k = []
                for kc in range(KC):
                    t1 = wpool.tile([128, H], F32R, tag=f"w1k{kc}")
                    nc.sync.dma_start(t1[:], W1[lvl][kc * 128:(kc + 1) * 128, :])
                    w1k.append(t1)
                for kc in range(KC):
                    t2 = wpool.tile([128, H], F32R, tag=f"w2k{kc}")
                    nc.sync.dma_start(t2[:], W2[lvl][kc * 128:(kc + 1) * 128, :])
                    w2k.append(t2)
                w3t = bpool.tile([128, KC], F32R, tag="w3")
                nc.sync.dma_start(w3t[:], W3[lvl].rearrange("(kc p) o -> p (kc o)", p=128))
                b1t = bpool.tile([128, H // 128], F32, tag="b1")
                nc.sync.dma_start(b1t[:], b1[lvl].rearrange("(mc p) -> p mc", p=128))
                b2t = bpool.tile([128, H // 128], F32, tag="b2")
                nc.sync.dma_start(b2t[:], b2[lvl].rearrange("(mc p) -> p mc", p=128))
                b3t = bpool.tile([1, 1], F32, tag="b3")
                nc.sync.dma_start(b3t[:], b3[lvl:lvl + 1, :])

                for rt in _row_tiles(cap):
                    x_t = xpool.tile([128, KC, rt], F32R, tag="x")
                    nc.sync.dma_start(x_t[:], xT_r[:, :, off:off + rt])

                    h1 = hpool.tile([128, H // 128, rt], F32R, tag="h1")
                    for mc in range(H // 128):
                        acc = ps.tile([128, rt], F32)
                        for kc in range(KC):
                            nc.tensor.matmul(
                                acc[:], w1k[kc][:, mc * 128:(mc + 1) * 128],
                                x_t[:, kc, :], start=(kc == 0), stop=(kc == KC - 1))
                        nc.vector.tensor_scalar(
                            h1[:, mc, :], acc[:], b1t[:, mc:mc + 1], 0.0, ADD, MAX)

                    h2 = hpool.tile([128, H // 128, rt], F32R, tag="h2")
                    for mc in range(H // 128):
                        acc = ps.tile([128, rt], F32)
                        for kc in range(H // 128):
                            nc.tensor.matmul(
                                acc[:], w2k[kc][:, mc * 128:(mc + 1) * 128],
                                h1[:, kc, :], start=(kc == 0), stop=(kc == H // 128 - 1))
                        nc.vector.tensor_scalar(
                            h2[:, mc, :], acc[:], b2t[:, mc:mc + 1], 0.0, ADD, MAX)

                    acc3 = ps3.tile([1, rt], F32)
                    for kc in range(H // 128):
                        nc.tensor.matmul(acc3[:], w3t[:, kc:kc + 1], h2[:, kc, :],
                                         start=(kc == 0), stop=(kc == H // 128 - 1))
                    o_t = opool.tile([1, rt], F32, tag="o")
                    nc.vector.tensor_scalar(o_t[:], acc3[:], b3t[:], None, ADD)
                    nc.sync.dma_start(out[:, off:off + rt], o_t[:])
                    off += rt
    nc.compile()
    return nc


def kernel(x, levels, W1, b1, W2, b2, W3, b3):
    global LAST_RESULTS
    x = np.ascontiguousarray(np.asarray(x, dtype=np.float32))
    levels = np.asarray(levels)
    n = x.shape[0]

    # --- host-side routing: sort rows by level, deal evenly to cores ---
    order = np.argsort(levels, kind="stable")
    counts = np.bincount(np.asarray(levels, dtype=np.int64), minlength=L)[:L]

    # per-level capacity shared by all cores: ceil(max per-core count / 256)*256
    caps = []
    for lvl in range(L):
        per_core_max = -(-int(counts[lvl]) // NC)
        caps.append(-(-per_core_max // 256) * 256 if per_core_max else 0)
    r_core = sum(caps)

    # per-core padded index lists + validity masks
    idx = np.zeros((NC, r_core), dtype=np.int64)
    valid = np.zeros((NC, r_core), dtype=bool)
    lvl_start = np.concatenate([[0], np.cumsum(counts)])
    seg_off = 0
    for lvl in range(L):
        rows = order[lvl_start[lvl]:lvl_start[lvl + 1]]
        nl = len(rows)
        q, rem = divmod(nl, NC)
        start = 0
        for c in range(NC):
            cnt = q + (1 if c < rem else 0)
            idx[c, seg_off:seg_off + cnt] = rows[start:start + cnt]
            valid[c, seg_off:seg_off + cnt] = True
            start += cnt
        seg_off += caps[lvl]

    key = tuple(caps)
    nc = _PROGRAM_CACHE.get(key)
    if nc is None:
        nc = _build_program(caps)
        _PROGRAM_CACHE[key] = nc

    in_maps = []
    for c in range(NC):
        xTc = np.ascontiguousarray(x[idx[c]].T)  # [D, r_core]
        in_maps.append({
            "xT": xTc,
            "W1": np.asarray(W1, dtype=np.float32),
            "W2": np.asarray(W2, dtype=np.float32),
            "W3": np.asarray(W3, dtype=np.float32),
            "b1": np.asarray(b1, dtype=np.float32),
            "b2": np.asarray(b2, dtype=np.float32),
            "b3": np.asarray(b3, dtype=np.float32),
        })

    trace = bool(os.environ.get("BASS_KERNEL_TRACE"))
    res = run_bass_kernel_spmd(nc, in_maps, core_ids=list(range(NC)), trace=trace)
    LAST_RESULTS = res

    result = np.zeros((n, 1), dtype=np.float32)
    for c in range(NC):
        o = np.asarray(res.results[c]["out"]).reshape(-1)
        result[idx[c][valid[c]], 0] = o[valid[c]]
    return result


# revision 5
# speedup vs baseline: 1.0911x; 1.0911x over previous
"""Trainium2 Bass kernel for nn_DAInsHead (moe_routing).

Per-row hard-routed 3-layer MLP: rows with levels[i]==l get
    out[i] = W3[l].T @ relu(W2[l].T @ relu(W1[l].T @ x[i] + b1[l]) + b2[l]) + b3[l]

Strategy (vs the reference's dense 4x-redundant masked compute):
  * Host: stable-sort rows by level, deal each level's rows evenly to the 8
    cores, pad each (core, level) segment to a shared per-level capacity
    (multiple of 256), and transpose to feature-major xT [D, R_core] so the
    device needs no on-chip transposes.
  * Device (identical SPMD program on 8 cores): for each level, keep that
    level's W1/W2 resident in SBUF (f32r) and stream row tiles of 512:
    L1/L2 are K=8-chunk accumulated 128x128x512 f32r matmuls (full PE rate,
    ~1 cycle/row) with DVE relu+bias eviction; L3 is a K-chunked matvec.
  * Host: scatter per-core outputs back to original row order.

f32r (fp32 data issued to the PE in reduced-precision streaming mode) gives
~1.5e-4 scale-relative error per 1024-deep matmul vs 2e-3 for bf16, at 4x
the throughput of plain fp32 matmul.
"""
import os
import sys

sys.path.insert(0, "/opt/trn_rl_repo")

import numpy as np

import concourse.bacc as bacc
import concourse.mybir as mybir
import concourse.tile as tile
from concourse.bass_utils import run_bass_kernel_spmd

F32 = mybir.dt.float32
F32R = mybir.dt.float32r
ADD = mybir.AluOpType.add
MAX = mybir.AluOpType.max

NC = 8          # cores
L = 4           # levels
D = 1024        # in features
H = 1024        # hidden
KC = D // 128   # contraction chunks

LAST_RESULTS = None       # BassKernelResults of the most recent run (for test.py)
_PROGRAM_CACHE = {}


def _row_tiles(c):
    """Split a per-level capacity (multiple of 256) into row-tile sizes."""
    tiles = [512] * (c // 512)
    if c % 512:
        tiles.append(c % 512)
    return tiles


def _build_program(caps):
    """Build + compile the SPMD program for per-level capacities `caps`."""
    r_core = sum(caps)
    nc = bacc.Bacc("TRN2", target_bir_lowering=False, debug=False, num_devices=NC)
    xT = nc.dram_tensor("xT", [D, r_core], F32R, kind="ExternalInput")
    W1 = nc.dram_tensor("W1", [L, D, H], F32R, kind="ExternalInput")
    W2 = nc.dram_tensor("W2", [L, H, H], F32R, kind="ExternalInput")
    W3 = nc.dram_tensor("W3", [L, H, 1], F32R, kind="ExternalInput")
    b1 = nc.dram_tensor("b1", [L, H], F32, kind="ExternalInput")
    b2 = nc.dram_tensor("b2", [L, H], F32, kind="ExternalInput")
    b3 = nc.dram_tensor("b3", [L, 1], F32, kind="ExternalInput")
    out = nc.dram_tensor("out", [1, r_core], F32, kind="ExternalOutput")

    xT_r = xT.rearrange("(kc p) r -> p kc r", p=128)  # [128, KC, r_core]

    with tile.TileContext(nc) as tc:
        with (
            tc.tile_pool(name="wpool", bufs=2) as wpool,
            tc.tile_pool(name="bpool", bufs=2) as bpool,
            tc.tile_pool(name="xpool", bufs=2) as xpool,
            tc.tile_pool(name="hpool", bufs=1) as hpool,
            tc.tile_pool(name="opool", bufs=3) as opool,
            tc.tile_pool(name="ps", bufs=6, space="PSUM") as ps,
            tc.tile_pool(name="ps3", bufs=2, space="PSUM") as ps3,
        ):
            off = 0
            for lvl in range(L):
                cap = caps[lvl]
                if cap == 0:
                    continue
                # For level 0, issue the first row-tile's x DMA before the
                # weight DMAs so the PE can start as soon as the first weight
                # chunk lands instead of waiting behind 8.5MB of weights.
                pre_x = None
                if lvl == 0:
                    rt0 = _row_tiles(cap)[0]
                    pre_x = xpool.tile([128, KC, rt0], F32R, tag="x")
                    nc.sync.dma_start(pre_x[:], xT_r[:, :, 0:rt0])
                # Per-kc weight tiles so the first matmuls only wait on the
                # first 512KB of weight DMA, and level l+1 prefetch
                # double-buffers against level l (bufs=2 per tag).
                w1k = []
                w2k = []
                for kc in range(KC):
                    t1 = wpool.tile([128, H], F32R, tag=f"w1k{kc}")
                    nc.sync.dma_start(t1[:], W1[lvl][kc * 128:(kc + 1) * 128, :])
                    w1k.append(t1)
                for kc in range(KC):
                    t2 = wpool.tile([128, H], F32R, tag=f"w2k{kc}")
                    nc.sync.dma_start(t2[:], W2[lvl][kc * 128:(kc + 1) * 128, :])
                    w2k.append(t2)
                w3t = bpool.tile([128, KC], F32R, tag="w3")
                nc.sync.dma_start(w3t[:], W3[lvl].rearrange("(kc p) o -> p (kc o)", p=128))
                b1t = bpool.tile([128, H // 128], F32, tag="b1")
                nc.sync.dma_start(b1t[:], b1[lvl].rearrange("(mc p) -> p mc", p=128))
                b2t = bpool.tile([128, H // 128], F32, tag="b2")
                nc.sync.dma_start(b2t[:], b2[lvl].rearrange("(mc p) -> p mc", p=128))
                b3t = bpool.tile([1, 1], F32, tag="b3")
                nc.sync.dma_start(b3t[:], b3[lvl:lvl + 1, :])

                for ti, rt in enumerate(_row_tiles(cap)):
                    if pre_x is not None and ti == 0:
                        x_t = pre_x
                    else:
                        x_t = xpool.tile([128, KC, rt], F32R, tag="x")
                        nc.sync.dma_start(x_t[:], xT_r[:, :, off:off + rt])

                    # L1 runs kc-outer in two 4-bank halves: the first matmul
                    # only depends on w1k[0] + x_t, so the PE ramps with the
                    # weight DMA stream instead of waiting for all of W1.
                    h1 = hpool.tile([128, H // 128, rt], F32R, tag="h1")
                    for half in range(2):
                        mcs = range(4 * half, 4 * half + 4)
                        accs = {mc: ps.tile([128, rt], F32, tag="acc", name="acc")
                                for mc in mcs}
                        for kc in range(KC):
                            for mc in mcs:
                                nc.tensor.matmul(
                                    accs[mc][:], w1k[kc][:, mc * 128:(mc + 1) * 128],
                                    x_t[:, kc, :], start=(kc == 0), stop=(kc == KC - 1))
                        for mc in mcs:
                            nc.vector.tensor_scalar(
                                h1[:, mc, :], accs[mc][:], b1t[:, mc:mc + 1], 0.0, ADD, MAX)

                    h2 = hpool.tile([128, H // 128, rt], F32R, tag="h2")
                    for mc in range(H // 128):
                        acc = ps.tile([128, rt], F32)
                        for kc in range(H // 128):
                            nc.tensor.matmul(
                                acc[:], w2k[kc][:, mc * 128:(mc + 1) * 128],
                                h1[:, kc, :], start=(kc == 0), stop=(kc == H // 128 - 1))
                        nc.vector.tensor_scalar(
                            h2[:, mc, :], acc[:], b2t[:, mc:mc + 1], 0.0, ADD, MAX)

                    acc3 = ps3.tile([1, rt], F32)
                    for kc in range(H // 128):
                        nc.tensor.matmul(acc3[:], w3t[:, kc:kc + 1], h2[:, kc, :],
                                         start=(kc == 0), stop=(kc == H // 128 - 1))
                    o_t = opool.tile([1, rt], F32, tag="o")
                    nc.vector.tensor_scalar(o_t[:], acc3[:], b3t[:], None, ADD)
                    nc.sync.dma_start(out[:, off:off + rt], o_t[:])
                    off += rt
    nc.compile()
    return nc


def kernel(x, levels, W1, b1, W2, b2, W3, b3):
    global LAST_RESULTS
    x = np.ascontiguousarray(np.asarray(x, dtype=np.float32))
    levels = np.asarray(levels)
    n = x.shape[0]

    # --- host-side routing: sort rows by level, deal evenly to cores ---
    order = np.argsort(levels, kind="stable")
    counts = np.bincount(np.asarray(levels, dtype=np.int64), minlength=L)[:L]

    # per-level capacity shared by all cores: ceil(max per-core count / 256)*256
    caps = []
    for lvl in range(L):
        per_core_max = -(-int(counts[lvl]) // NC)
        caps.append(-(-per_core_max // 256) * 256 if per_core_max else 0)
    r_core = sum(caps)

    # per-core padded index lists + validity masks
    idx = np.zeros((NC, r_core), dtype=np.int64)
    valid = np.zeros((NC, r_core), dtype=bool)
    lvl_start = np.concatenate([[0], np.cumsum(counts)])
    seg_off = 0
    for lvl in range(L):
        rows = order[lvl_start[lvl]:lvl_start[lvl + 1]]
        nl = len(rows)
        q, rem = divmod(nl, NC)
        start = 0
        for c in range(NC):
            cnt = q + (1 if c < rem else 0)
            idx[c, seg_off:seg_off + cnt] = rows[start:start + cnt]
            valid[c, seg_off:seg_off + cnt] = True
            start += cnt
        seg_off += caps[lvl]

    key = tuple(caps)
    nc = _PROGRAM_CACHE.get(key)
    if nc is None:
        nc = _build_program(caps)
        _PROGRAM_CACHE[key] = nc

    in_maps = []
    for c in range(NC):
        xTc = np.ascontiguousarray(x[idx[c]].T)  # [D, r_core]
        in_maps.append({
            "xT": xTc,
            "W1": np.asarray(W1, dtype=np.float32),
            "W2": np.asarray(W2, dtype=np.float32),
            "W3": np.asarray(W3, dtype=np.float32),
            "b1": np.asarray(b1, dtype=np.float32),
            "b2": np.asarray(b2, dtype=np.float32),
            "b3": np.asarray(b3, dtype=np.float32),
        })

    trace = bool(os.environ.get("BASS_KERNEL_TRACE"))
    res = run_bass_kernel_spmd(nc, in_maps, core_ids=list(range(NC)), trace=trace)
    LAST_RESULTS = res

    result = np.zeros((n, 1), dtype=np.float32)
    for c in range(NC):
        o = np.asarray(res.results[c]["out"]).reshape(-1)
        result[idx[c][valid[c]], 0] = o[valid[c]]
    return result


# revision 6
# speedup vs baseline: 1.0916x; 1.0005x over previous
"""Trainium2 Bass kernel for nn_DAInsHead (moe_routing).

Per-row hard-routed 3-layer MLP: rows with levels[i]==l get
    out[i] = W3[l].T @ relu(W2[l].T @ relu(W1[l].T @ x[i] + b1[l]) + b2[l]) + b3[l]

Strategy (vs the reference's dense 4x-redundant masked compute):
  * Host: stable-sort rows by level, deal each level's rows evenly to the 8
    cores, pad each (core, level) segment to a shared per-level capacity
    (multiple of 256), and transpose to feature-major xT [D, R_core] so the
    device needs no on-chip transposes.
  * Device (identical SPMD program on 8 cores): for each level, keep that
    level's W1/W2 resident in SBUF (f32r) and stream row tiles of 512:
    L1/L2 are K=8-chunk accumulated 128x128x512 f32r matmuls (full PE rate,
    ~1 cycle/row) with DVE relu+bias eviction; L3 is a K-chunked matvec.
  * Host: scatter per-core outputs back to original row order.

f32r (fp32 data issued to the PE in reduced-precision streaming mode) gives
~1.5e-4 scale-relative error per 1024-deep matmul vs 2e-3 for bf16, at 4x
the throughput of plain fp32 matmul.
"""
import os
import sys

sys.path.insert(0, "/opt/trn_rl_repo")

import numpy as np

import concourse.bacc as bacc
import concourse.mybir as mybir
import concourse.tile as tile
from concourse.bass_utils import run_bass_kernel_spmd

F32 = mybir.dt.float32
F32R = mybir.dt.float32r
ADD = mybir.AluOpType.add
MAX = mybir.AluOpType.max

NC = 8          # cores
L = 4           # levels
D = 1024        # in features
H = 1024        # hidden
KC = D // 128   # contraction chunks

LAST_RESULTS = None       # BassKernelResults of the most recent run (for test.py)
_PROGRAM_CACHE = {}


def _row_tiles(c):
    """Split a per-level capacity (multiple of 256) into row-tile sizes."""
    tiles = [512] * (c // 512)
    if c % 512:
        tiles.append(c % 512)
    return tiles


def _build_program(caps):
    """Build + compile the SPMD program for per-level capacities `caps`."""
    r_core = sum(caps)
    nc = bacc.Bacc("TRN2", target_bir_lowering=False, debug=False, num_devices=NC)
    xT = nc.dram_tensor("xT", [D, r_core], F32R, kind="ExternalInput")
    W1 = nc.dram_tensor("W1", [L, D, H], F32R, kind="ExternalInput")
    W2 = nc.dram_tensor("W2", [L, H, H], F32R, kind="ExternalInput")
    W3 = nc.dram_tensor("W3", [L, H, 1], F32R, kind="ExternalInput")
    b1 = nc.dram_tensor("b1", [L, H], F32, kind="ExternalInput")
    b2 = nc.dram_tensor("b2", [L, H], F32, kind="ExternalInput")
    b3 = nc.dram_tensor("b3", [L, 1], F32, kind="ExternalInput")
    out = nc.dram_tensor("out", [1, r_core], F32, kind="ExternalOutput")

    xT_r = xT.rearrange("(kc p) r -> p kc r", p=128)  # [128, KC, r_core]

    with tile.TileContext(nc) as tc:
        with (
            tc.tile_pool(name="wpool", bufs=2) as wpool,
            tc.tile_pool(name="bpool", bufs=2) as bpool,
            tc.tile_pool(name="xpool", bufs=2) as xpool,
            tc.tile_pool(name="hpool", bufs=1) as hpool,
            tc.tile_pool(name="opool", bufs=3) as opool,
            tc.tile_pool(name="ps", bufs=6, space="PSUM") as ps,
            tc.tile_pool(name="ps3", bufs=2, space="PSUM") as ps3,
        ):
            off = 0
            for lvl in range(L):
                cap = caps[lvl]
                if cap == 0:
                    continue
                # For level 0, issue the first row-tile's x DMA before the
                # weight DMAs so the PE can start as soon as the first weight
                # chunk lands instead of waiting behind 8.5MB of weights.
                pre_x = None
                if lvl == 0:
                    rt0 = _row_tiles(cap)[0]
                    pre_x = xpool.tile([128, KC, rt0], F32R, tag="x")
                    nc.sync.dma_start(pre_x[:], xT_r[:, :, 0:rt0])
                # Per-kc weight tiles so the first matmuls only wait on the
                # first 512KB of weight DMA, and level l+1 prefetch
                # double-buffers against level l (bufs=2 per tag).
                w1k = []
                w2k = []
                for kc in range(KC):
                    t1 = wpool.tile([128, H], F32R, tag=f"w1k{kc}")
                    nc.sync.dma_start(t1[:], W1[lvl][kc * 128:(kc + 1) * 128, :])
                    w1k.append(t1)
                for kc in range(KC):
                    t2 = wpool.tile([128, H], F32R, tag=f"w2k{kc}")
                    nc.sync.dma_start(t2[:], W2[lvl][kc * 128:(kc + 1) * 128, :])
                    w2k.append(t2)
                w3t = bpool.tile([128, KC], F32R, tag="w3")
                nc.sync.dma_start(w3t[:], W3[lvl].rearrange("(kc p) o -> p (kc o)", p=128))
                b1t = bpool.tile([128, H // 128], F32, tag="b1")
                nc.sync.dma_start(b1t[:], b1[lvl].rearrange("(mc p) -> p mc", p=128))
                b2t = bpool.tile([128, H // 128], F32, tag="b2")
                nc.sync.dma_start(b2t[:], b2[lvl].rearrange("(mc p) -> p mc", p=128))
                b3t = bpool.tile([1, 1], F32, tag="b3")
                nc.sync.dma_start(b3t[:], b3[lvl:lvl + 1, :])

                for ti, rt in enumerate(_row_tiles(cap)):
                    if pre_x is not None and ti == 0:
                        x_t = pre_x
                    else:
                        x_t = xpool.tile([128, KC, rt], F32R, tag="x")
                        nc.sync.dma_start(x_t[:], xT_r[:, :, off:off + rt])

                    # L1 runs kc-outer in two 4-bank halves: the first matmul
                    # only depends on w1k[0] + x_t, so the PE ramps with the
                    # weight DMA stream instead of waiting for all of W1.
                    h1 = hpool.tile([128, H // 128, rt], F32R, tag="h1")
                    for half in range(2):
                        mcs = range(4 * half, 4 * half + 4)
                        accs = {mc: ps.tile([128, rt], F32, tag="acc", name="acc")
                                for mc in mcs}
                        for kc in range(KC):
                            for mc in mcs:
                                nc.tensor.matmul(
                                    accs[mc][:], w1k[kc][:, mc * 128:(mc + 1) * 128],
                                    x_t[:, kc, :], start=(kc == 0), stop=(kc == KC - 1))
                        for mc in mcs:
                            nc.vector.tensor_scalar(
                                h1[:, mc, :], accs[mc][:], b1t[:, mc:mc + 1], 0.0, ADD, MAX)

                    h2 = hpool.tile([128, H // 128, rt], F32R, tag="h2")
                    for half in range(2):
                        mcs = range(4 * half, 4 * half + 4)
                        accs = {mc: ps.tile([128, rt], F32, tag="acc", name="acc")
                                for mc in mcs}
                        for kc in range(H // 128):
                            for mc in mcs:
                                nc.tensor.matmul(
                                    accs[mc][:], w2k[kc][:, mc * 128:(mc + 1) * 128],
                                    h1[:, kc, :], start=(kc == 0), stop=(kc == H // 128 - 1))
                        for mc in mcs:
                            nc.vector.tensor_scalar(
                                h2[:, mc, :], accs[mc][:], b2t[:, mc:mc + 1], 0.0, ADD, MAX)

                    acc3 = ps3.tile([1, rt], F32)
                    for kc in range(H // 128):
                        nc.tensor.matmul(acc3[:], w3t[:, kc:kc + 1], h2[:, kc, :],
                                         start=(kc == 0), stop=(kc == H // 128 - 1))
                    o_t = opool.tile([1, rt], F32, tag="o")
                    nc.vector.tensor_scalar(o_t[:], acc3[:], b3t[:], None, ADD)
                    nc.sync.dma_start(out[:, off:off + rt], o_t[:])
                    off += rt
    nc.compile()
    return nc


def kernel(x, levels, W1, b1, W2, b2, W3, b3):
    global LAST_RESULTS
    x = np.ascontiguousarray(np.asarray(x, dtype=np.float32))
    levels = np.asarray(levels)
    n = x.shape[0]

    # --- host-side routing: sort rows by level, deal evenly to cores ---
    order = np.argsort(levels, kind="stable")
    counts = np.bincount(np.asarray(levels, dtype=np.int64), minlength=L)[:L]

    # per-level capacity shared by all cores: ceil(max per-core count / 256)*256
    caps = []
    for lvl in range(L):
        per_core_max = -(-int(counts[lvl]) // NC)
        caps.append(-(-per_core_max // 256) * 256 if per_core_max else 0)
    r_core = sum(caps)

    # per-core padded index lists + validity masks
    idx = np.zeros((NC, r_core), dtype=np.int64)
    valid = np.zeros((NC, r_core), dtype=bool)
    lvl_start = np.concatenate([[0], np.cumsum(counts)])
    seg_off = 0
    for lvl in range(L):
        rows = order[lvl_start[lvl]:lvl_start[lvl + 1]]
        nl = len(rows)
        q, rem = divmod(nl, NC)
        start = 0
        for c in range(NC):
            cnt = q + (1 if c < rem else 0)
            idx[c, seg_off:seg_off + cnt] = rows[start:start + cnt]
            valid[c, seg_off:seg_off + cnt] = True
            start += cnt
        seg_off += caps[lvl]

    key = tuple(caps)
    nc = _PROGRAM_CACHE.get(key)
    if nc is None:
        nc = _build_program(caps)
        _PROGRAM_CACHE[key] = nc

    in_maps = []
    for c in range(NC):
        xTc = np.ascontiguousarray(x[idx[c]].T)  # [D, r_core]
        in_maps.append({
            "xT": xTc,
            "W1": np.asarray(W1, dtype=np.float32),
            "W2": np.asarray(W2, dtype=np.float32),
            "W3": np.asarray(W3, dtype=np.float32),
            "b1": np.asarray(b1, dtype=np.float32),
            "b2": np.asarray(b2, dtype=np.float32),
            "b3": np.asarray(b3, dtype=np.float32),
        })

    trace = bool(os.environ.get("BASS_KERNEL_TRACE"))
    res = run_bass_kernel_spmd(nc, in_maps, core_ids=list(range(NC)), trace=trace)
    LAST_RESULTS = res

    result = np.zeros((n, 1), dtype=np.float32)
    for c in range(NC):
        o = np.asarray(res.results[c]["out"]).reshape(-1)
        result[idx[c][valid[c]], 0] = o[valid[c]]
    return result


# revision 7
# speedup vs baseline: 1.1000x; 1.0077x over previous
"""Trainium2 Bass kernel for nn_DAInsHead (moe_routing).

Per-row hard-routed 3-layer MLP: rows with levels[i]==l get
    out[i] = W3[l].T @ relu(W2[l].T @ relu(W1[l].T @ x[i] + b1[l]) + b2[l]) + b3[l]

Strategy (vs the reference's dense 4x-redundant masked compute):
  * Host: stable-sort rows by level, deal each level's rows evenly to the 8
    cores, pad each (core, level) segment to a shared per-level capacity
    (multiple of 256), and transpose to feature-major xT [D, R_core] so the
    device needs no on-chip transposes.
  * Device (identical SPMD program on 8 cores): for each level, keep that
    level's W1/W2 resident in SBUF (f32r) and stream row tiles of 512:
    L1/L2 are K=8-chunk accumulated 128x128x512 f32r matmuls (full PE rate,
    ~1 cycle/row) with DVE relu+bias eviction; L3 is a K-chunked matvec.
  * Host: scatter per-core outputs back to original row order.

f32r (fp32 data issued to the PE in reduced-precision streaming mode) gives
~1.5e-4 scale-relative error per 1024-deep matmul vs 2e-3 for bf16, at 4x
the throughput of plain fp32 matmul.
"""
import os
import sys

sys.path.insert(0, "/opt/trn_rl_repo")

import numpy as np

import concourse.bacc as bacc
import concourse.mybir as mybir
import concourse.tile as tile
from concourse.bass_utils import run_bass_kernel_spmd

F32 = mybir.dt.float32
F32R = mybir.dt.float32r
ADD = mybir.AluOpType.add
MAX = mybir.AluOpType.max

NC = 8          # cores
L = 4           # levels
D = 1024        # in features
H = 1024        # hidden
KC = D // 128   # contraction chunks

LAST_RESULTS = None       # BassKernelResults of the most recent run (for test.py)
_PROGRAM_CACHE = {}


def _row_tiles(c):
    """Split a per-level capacity (multiple of 256) into row-tile sizes."""
    tiles = [512] * (c // 512)
    if c % 512:
        tiles.append(c % 512)
    return tiles


def _build_program(caps):
    """Build + compile the SPMD program for per-level capacities `caps`."""
    r_core = sum(caps)
    nc = bacc.Bacc("TRN2", target_bir_lowering=False, debug=False, num_devices=NC)
    xT = nc.dram_tensor("xT", [D, r_core], F32R, kind="ExternalInput")
    W1 = nc.dram_tensor("W1", [L, D, H], F32R, kind="ExternalInput")
    W2 = nc.dram_tensor("W2", [L, H, H], F32R, kind="ExternalInput")
    W3 = nc.dram_tensor("W3", [L, H, 1], F32R, kind="ExternalInput")
    b1 = nc.dram_tensor("b1", [L, H], F32, kind="ExternalInput")
    b2 = nc.dram_tensor("b2", [L, H], F32, kind="ExternalInput")
    b3 = nc.dram_tensor("b3", [L, 1], F32, kind="ExternalInput")
    out = nc.dram_tensor("out", [1, r_core], F32, kind="ExternalOutput")

    xT_r = xT.rearrange("(kc p) r -> p kc r", p=128)  # [128, KC, r_core]

    with tile.TileContext(nc) as tc:
        with (
            tc.tile_pool(name="wpool", bufs=2) as wpool,
            tc.tile_pool(name="bpool", bufs=2) as bpool,
            tc.tile_pool(name="xpool", bufs=2) as xpool,
            tc.tile_pool(name="hpool", bufs=1) as hpool,
            tc.tile_pool(name="opool", bufs=3) as opool,
            tc.tile_pool(name="ps", bufs=6, space="PSUM") as ps,
            tc.tile_pool(name="ps3", bufs=2, space="PSUM") as ps3,
        ):
            off = 0
            for lvl in range(L):
                cap = caps[lvl]
                if cap == 0:
                    continue
                # For level 0, issue the first row-tile's x DMA before the
                # weight DMAs so the PE can start as soon as the first weight
                # chunk lands instead of waiting behind 8.5MB of weights.
                pre_x = None
                if lvl == 0:
                    rt0 = _row_tiles(cap)[0]
                    pre_x = xpool.tile([128, KC, rt0], F32R, tag="x")
                    nc.sync.dma_start(pre_x[:], xT_r[:, :, 0:rt0])
                # Tiny bias/W3 tiles first so evictions never wait behind the
                # 8MB of W1/W2 weight DMA.
                w3t = bpool.tile([128, KC], F32R, tag="w3")
                nc.sync.dma_start(w3t[:], W3[lvl].rearrange("(kc p) o -> p (kc o)", p=128))
                b1t = bpool.tile([128, H // 128], F32, tag="b1")
                nc.sync.dma_start(b1t[:], b1[lvl].rearrange("(mc p) -> p mc", p=128))
                b2t = bpool.tile([128, H // 128], F32, tag="b2")
                nc.sync.dma_start(b2t[:], b2[lvl].rearrange("(mc p) -> p mc", p=128))
                b3t = bpool.tile([1, 1], F32, tag="b3")
                nc.sync.dma_start(b3t[:], b3[lvl:lvl + 1, :])
                # Per-kc weight tiles so the first matmuls only wait on the
                # first 512KB of weight DMA, and level l+1 prefetch
                # double-buffers against level l (bufs=2 per tag).
                w1k = []
                w2k = []
                for kc in range(KC):
                    t1 = wpool.tile([128, H], F32R, tag=f"w1k{kc}")
                    nc.sync.dma_start(t1[:], W1[lvl][kc * 128:(kc + 1) * 128, :])
                    w1k.append(t1)
                for kc in range(KC):
                    t2 = wpool.tile([128, H], F32R, tag=f"w2k{kc}")
                    nc.sync.dma_start(t2[:], W2[lvl][kc * 128:(kc + 1) * 128, :])
                    w2k.append(t2)

                for ti, rt in enumerate(_row_tiles(cap)):
                    if pre_x is not None and ti == 0:
                        x_t = pre_x
                    else:
                        x_t = xpool.tile([128, KC, rt], F32R, tag="x")
                        nc.sync.dma_start(x_t[:], xT_r[:, :, off:off + rt])

                    # L1 runs kc-outer in two 4-bank halves: the first matmul
                    # only depends on w1k[0] + x_t, so the PE ramps with the
                    # weight DMA stream instead of waiting for all of W1.
                    h1 = hpool.tile([128, H // 128, rt], F32R, tag="h1")
                    for half in range(2):
                        mcs = range(4 * half, 4 * half + 4)
                        accs = {mc: ps.tile([128, rt], F32, tag="acc", name="acc")
                                for mc in mcs}
                        for kc in range(KC):
                            for mc in mcs:
                                nc.tensor.matmul(
                                    accs[mc][:], w1k[kc][:, mc * 128:(mc + 1) * 128],
                                    x_t[:, kc, :], start=(kc == 0), stop=(kc == KC - 1))
                        for mc in mcs:
                            nc.vector.tensor_scalar(
                                h1[:, mc, :], accs[mc][:], b1t[:, mc:mc + 1], 0.0, ADD, MAX)

                    h2 = hpool.tile([128, H // 128, rt], F32R, tag="h2")
                    for half in range(2):
                        mcs = range(4 * half, 4 * half + 4)
                        accs = {mc: ps.tile([128, rt], F32, tag="acc", name="acc")
                                for mc in mcs}
                        for kc in range(H // 128):
                            for mc in mcs:
                                nc.tensor.matmul(
                                    accs[mc][:], w2k[kc][:, mc * 128:(mc + 1) * 128],
                                    h1[:, kc, :], start=(kc == 0), stop=(kc == H // 128 - 1))
                        for mc in mcs:
                            nc.vector.tensor_scalar(
                                h2[:, mc, :], accs[mc][:], b2t[:, mc:mc + 1], 0.0, ADD, MAX)

                    acc3 = ps3.tile([1, rt], F32)
                    for kc in range(H // 128):
                        nc.tensor.matmul(acc3[:], w3t[:, kc:kc + 1], h2[:, kc, :],
                                         start=(kc == 0), stop=(kc == H // 128 - 1))
                    o_t = opool.tile([1, rt], F32, tag="o")
                    nc.vector.tensor_scalar(o_t[:], acc3[:], b3t[:], None, ADD)
                    nc.sync.dma_start(out[:, off:off + rt], o_t[:])
                    off += rt
    nc.compile()
    return nc


def kernel(x, levels, W1, b1, W2, b2, W3, b3):
    global LAST_RESULTS
    x = np.ascontiguousarray(np.asarray(x, dtype=np.float32))
    levels = np.asarray(levels)
    n = x.shape[0]

    # --- host-side routing: sort rows by level, deal evenly to cores ---
    order = np.argsort(levels, kind="stable")
    counts = np.bincount(np.asarray(levels, dtype=np.int64), minlength=L)[:L]

    # per-level capacity shared by all cores: ceil(max per-core count / 256)*256
    caps = []
    for lvl in range(L):
        per_core_max = -(-int(counts[lvl]) // NC)
        caps.append(-(-per_core_max // 256) * 256 if per_core_max else 0)
    r_core = sum(caps)

    # per-core padded index lists + validity masks
    idx = np.zeros((NC, r_core), dtype=np.int64)
    valid = np.zeros((NC, r_core), dtype=bool)
    lvl_start = np.concatenate([[0], np.cumsum(counts)])
    seg_off = 0
    for lvl in range(L):
        rows = order[lvl_start[lvl]:lvl_start[lvl + 1]]
        nl = len(rows)
        q, rem = divmod(nl, NC)
        start = 0
        for c in range(NC):
            cnt = q + (1 if c < rem else 0)
            idx[c, seg_off:seg_off + cnt] = rows[start:start + cnt]
            valid[c, seg_off:seg_off + cnt] = True
            start += cnt
        seg_off += caps[lvl]

    key = tuple(caps)
    nc = _PROGRAM_CACHE.get(key)
    if nc is None:
        nc = _build_program(caps)
        _PROGRAM_CACHE[key] = nc

    in_maps = []
    for c in range(NC):
        xTc = np.ascontiguousarray(x[idx[c]].T)  # [D, r_core]
        in_maps.append({
            "xT": xTc,
            "W1": np.asarray(W1, dtype=np.float32),
            "W2": np.asarray(W2, dtype=np.float32),
            "W3": np.asarray(W3, dtype=np.float32),
            "b1": np.asarray(b1, dtype=np.float32),
            "b2": np.asarray(b2, dtype=np.float32),
            "b3": np.asarray(b3, dtype=np.float32),
        })

    trace = bool(os.environ.get("BASS_KERNEL_TRACE"))
    res = run_bass_kernel_spmd(nc, in_maps, core_ids=list(range(NC)), trace=trace)
    LAST_RESULTS = res

    result = np.zeros((n, 1), dtype=np.float32)
    for c in range(NC):
        o = np.asarray(res.results[c]["out"]).reshape(-1)
        result[idx[c][valid[c]], 0] = o[valid[c]]
    return result


# revision 9
# speedup vs baseline: 1.1015x; 1.0014x over previous
"""Trainium2 Bass kernel for nn_DAInsHead (moe_routing).

Per-row hard-routed 3-layer MLP: rows with levels[i]==l get
    out[i] = W3[l].T @ relu(W2[l].T @ relu(W1[l].T @ x[i] + b1[l]) + b2[l]) + b3[l]

Strategy (vs the reference's dense 4x-redundant masked compute):
  * Host: stable-sort rows by level, deal each level's rows evenly to the 8
    cores, pad each (core, level) segment to a shared per-level capacity
    (multiple of 256), and transpose to feature-major xT [D, R_core] so the
    device needs no on-chip transposes.
  * Device (identical SPMD program on 8 cores): for each level, keep that
    level's W1/W2 resident in SBUF (f32r) and stream row tiles of 512:
    L1/L2 are K=8-chunk accumulated 128x128x512 f32r matmuls (full PE rate,
    ~1 cycle/row) with DVE relu+bias eviction; L3 is a K-chunked matvec.
  * Host: scatter per-core outputs back to original row order.

f32r (fp32 data issued to the PE in reduced-precision streaming mode) gives
~1.5e-4 scale-relative error per 1024-deep matmul vs 2e-3 for bf16, at 4x
the throughput of plain fp32 matmul.
"""
import os
import sys

sys.path.insert(0, "/opt/trn_rl_repo")

import numpy as np

import concourse.bacc as bacc
import concourse.mybir as mybir
import concourse.tile as tile
from concourse.bass_utils import run_bass_kernel_spmd

F32 = mybir.dt.float32
F32R = mybir.dt.float32r
ADD = mybir.AluOpType.add
MAX = mybir.AluOpType.max

NC = 8          # cores
L = 4           # levels
D = 1024        # in features
H = 1024        # hidden
KC = D // 128   # contraction chunks

LAST_RESULTS = None       # BassKernelResults of the most recent run (for test.py)
_PROGRAM_CACHE = {}


def _row_tiles(c):
    """Split a per-level capacity (multiple of 256) into row-tile sizes."""
    tiles = [512] * (c // 512)
    if c % 512:
        tiles.append(c % 512)
    return tiles


def _build_program(caps):
    """Build + compile the SPMD program for per-level capacities `caps`."""
    r_core = sum(caps)
    nc = bacc.Bacc("TRN2", target_bir_lowering=False, debug=False, num_devices=NC)
    xT = nc.dram_tensor("xT", [D, r_core], F32R, kind="ExternalInput")
    W1 = nc.dram_tensor("W1", [L, D, H], F32R, kind="ExternalInput")
    W2 = nc.dram_tensor("W2", [L, H, H], F32R, kind="ExternalInput")
    W3 = nc.dram_tensor("W3", [L, H, 1], F32R, kind="ExternalInput")
    b1 = nc.dram_tensor("b1", [L, H], F32, kind="ExternalInput")
    b2 = nc.dram_tensor("b2", [L, H], F32, kind="ExternalInput")
    b3 = nc.dram_tensor("b3", [L, 1], F32, kind="ExternalInput")
    out = nc.dram_tensor("out", [1, r_core], F32, kind="ExternalOutput")

    xT_r = xT.rearrange("(kc p) r -> p kc r", p=128)  # [128, KC, r_core]

    with tile.TileContext(nc) as tc:
        with (
            tc.tile_pool(name="wpool", bufs=2) as wpool,
            tc.tile_pool(name="bpool", bufs=2) as bpool,
            tc.tile_pool(name="xpool", bufs=2) as xpool,
            tc.tile_pool(name="hpool", bufs=1) as hpool,
            tc.tile_pool(name="opool", bufs=3) as opool,
            tc.tile_pool(name="ps", bufs=7, space="PSUM") as ps,
            tc.tile_pool(name="ps3", bufs=1, space="PSUM") as ps3,
        ):
            off = 0
            for lvl in range(L):
                cap = caps[lvl]
                if cap == 0:
                    continue
                # For level 0, issue the first row-tile's x DMA before the
                # weight DMAs so the PE can start as soon as the first weight
                # chunk lands instead of waiting behind 8.5MB of weights.
                pre_x = None
                if lvl == 0:
                    rt0 = _row_tiles(cap)[0]
                    pre_x = xpool.tile([128, KC, rt0], F32R, tag="x")
                    # per-kc chunk DMAs: if region-level deps are tracked the
                    # first matmul starts after just 256KB of x
                    for kc in range(KC):
                        nc.sync.dma_start(pre_x[:, kc, :], xT_r[:, kc, 0:rt0])
                # Tiny bias/W3 tiles first so evictions never wait behind the
                # 8MB of W1/W2 weight DMA.
                w3t = bpool.tile([128, KC], F32R, tag="w3")
                nc.sync.dma_start(w3t[:], W3[lvl].rearrange("(kc p) o -> p (kc o)", p=128))
                b1t = bpool.tile([128, H // 128], F32, tag="b1")
                nc.sync.dma_start(b1t[:], b1[lvl].rearrange("(mc p) -> p mc", p=128))
                b2t = bpool.tile([128, H // 128], F32, tag="b2")
                nc.sync.dma_start(b2t[:], b2[lvl].rearrange("(mc p) -> p mc", p=128))
                b3t = bpool.tile([1, 1], F32, tag="b3")
                nc.sync.dma_start(b3t[:], b3[lvl:lvl + 1, :])
                # Per-kc weight tiles so the first matmuls only wait on the
                # first 512KB of weight DMA, and level l+1 prefetch
                # double-buffers against level l (bufs=2 per tag).
                w1k = []
                w2k = []
                for kc in range(KC):
                    t1 = wpool.tile([128, H], F32R, tag=f"w1k{kc}")
                    nc.sync.dma_start(t1[:], W1[lvl][kc * 128:(kc + 1) * 128, :])
                    w1k.append(t1)
                for kc in range(KC):
                    t2 = wpool.tile([128, H], F32R, tag=f"w2k{kc}")
                    nc.sync.dma_start(t2[:], W2[lvl][kc * 128:(kc + 1) * 128, :])
                    w2k.append(t2)

                for ti, rt in enumerate(_row_tiles(cap)):
                    if pre_x is not None and ti == 0:
                        x_t = pre_x
                    else:
                        x_t = xpool.tile([128, KC, rt], F32R, tag="x")
                        nc.sync.dma_start(x_t[:], xT_r[:, :, off:off + rt])

                    # L1 runs kc-outer in two 4-bank halves: the first matmul
                    # only depends on w1k[0] + x_t, so the PE ramps with the
                    # weight DMA stream instead of waiting for all of W1.
                    h1 = hpool.tile([128, H // 128, rt], F32R, tag="h1")
                    for half in range(2):
                        mcs = range(4 * half, 4 * half + 4)
                        accs = {mc: ps.tile([128, rt], F32, tag="acc", name="acc")
                                for mc in mcs}
                        for kc in range(KC):
                            for mc in mcs:
                                nc.tensor.matmul(
                                    accs[mc][:], w1k[kc][:, mc * 128:(mc + 1) * 128],
                                    x_t[:, kc, :], start=(kc == 0), stop=(kc == KC - 1))
                        for mc in mcs:
                            nc.vector.tensor_scalar(
                                h1[:, mc, :], accs[mc][:], b1t[:, mc:mc + 1], 0.0, ADD, MAX)

                    h2 = hpool.tile([128, H // 128, rt], F32R, tag="h2")
                    for half in range(2):
                        mcs = range(4 * half, 4 * half + 4)
                        accs = {mc: ps.tile([128, rt], F32, tag="acc", name="acc")
                                for mc in mcs}
                        for kc in range(H // 128):
                            for mc in mcs:
                                nc.tensor.matmul(
                                    accs[mc][:], w2k[kc][:, mc * 128:(mc + 1) * 128],
                                    h1[:, kc, :], start=(kc == 0), stop=(kc == H // 128 - 1))
                        for mc in mcs:
                            nc.vector.tensor_scalar(
                                h2[:, mc, :], accs[mc][:], b2t[:, mc:mc + 1], 0.0, ADD, MAX)

                    acc3 = ps3.tile([1, rt], F32)
                    for kc in range(H // 128):
                        nc.tensor.matmul(acc3[:], w3t[:, kc:kc + 1], h2[:, kc, :],
                                         start=(kc == 0), stop=(kc == H // 128 - 1))
                    o_t = opool.tile([1, rt], F32, tag="o")
                    nc.vector.tensor_scalar(o_t[:], acc3[:], b3t[:], None, ADD)
                    nc.sync.dma_start(out[:, off:off + rt], o_t[:])
                    off += rt
    nc.compile()
    return nc


def kernel(x, levels, W1, b1, W2, b2, W3, b3):
    global LAST_RESULTS
    x = np.ascontiguousarray(np.asarray(x, dtype=np.float32))
    levels = np.asarray(levels)
    n = x.shape[0]

    # --- host-side routing: sort rows by level, deal evenly to cores ---
    order = np.argsort(levels, kind="stable")
    counts = np.bincount(np.asarray(levels, dtype=np.int64), minlength=L)[:L]

    # per-level capacity shared by all cores: ceil(max per-core count / 256)*256
    caps = []
    for lvl in range(L):
        per_core_max = -(-int(counts[lvl]) // NC)
        caps.append(-(-per_core_max // 256) * 256 if per_core_max else 0)
    r_core = sum(caps)

    # per-core padded index lists + validity masks
    idx = np.zeros((NC, r_core), dtype=np.int64)
    valid = np.zeros((NC, r_core), dtype=bool)
    lvl_start = np.concatenate([[0], np.cumsum(counts)])
    seg_off = 0
    for lvl in range(L):
        rows = order[lvl_start[lvl]:lvl_start[lvl + 1]]
        nl = len(rows)
        q, rem = divmod(nl, NC)
        start = 0
        for c in range(NC):
            cnt = q + (1 if c < rem else 0)
            idx[c, seg_off:seg_off + cnt] = rows[start:start + cnt]
            valid[c, seg_off:seg_off + cnt] = True
            start += cnt
        seg_off += caps[lvl]

    key = tuple(caps)
    nc = _PROGRAM_CACHE.get(key)
    if nc is None:
        nc = _build_program(caps)
        _PROGRAM_CACHE[key] = nc

    in_maps = []
    for c in range(NC):
        xTc = np.ascontiguousarray(x[idx[c]].T)  # [D, r_core]
        in_maps.append({
            "xT": xTc,
            "W1": np.asarray(W1, dtype=np.float32),
            "W2": np.asarray(W2, dtype=np.float32),
            "W3": np.asarray(W3, dtype=np.float32),
            "b1": np.asarray(b1, dtype=np.float32),
            "b2": np.asarray(b2, dtype=np.float32),
            "b3": np.asarray(b3, dtype=np.float32),
        })

    trace = bool(os.environ.get("BASS_KERNEL_TRACE"))
    res = run_bass_kernel_spmd(nc, in_maps, core_ids=list(range(NC)), trace=trace)
    LAST_RESULTS = res

    result = np.zeros((n, 1), dtype=np.float32)
    for c in range(NC):
        o = np.asarray(res.results[c]["out"]).reshape(-1)
        result[idx[c][valid[c]], 0] = o[valid[c]]
    return result


# revision 11
# speedup vs baseline: 1.1071x; 1.0051x over previous
"""Trainium2 Bass kernel for nn_DAInsHead (moe_routing).

Per-row hard-routed 3-layer MLP: rows with levels[i]==l get
    out[i] = W3[l].T @ relu(W2[l].T @ relu(W1[l].T @ x[i] + b1[l]) + b2[l]) + b3[l]

Strategy (vs the reference's dense 4x-redundant masked compute):
  * Host: stable-sort rows by level, deal each level's rows evenly to the 8
    cores, pad each (core, level) segment to a shared per-level capacity
    (multiple of 256), and transpose to feature-major xT [D, R_core] so the
    device needs no on-chip transposes.
  * Device (identical SPMD program on 8 cores): for each level, keep that
    level's W1/W2 resident in SBUF (f32r) and stream row tiles of 512:
    L1/L2 are K=8-chunk accumulated 128x128x512 f32r matmuls (full PE rate,
    ~1 cycle/row) with DVE relu+bias eviction; L3 is a K-chunked matvec.
  * Host: scatter per-core outputs back to original row order.

f32r (fp32 data issued to the PE in reduced-precision streaming mode) gives
~1.5e-4 scale-relative error per 1024-deep matmul vs 2e-3 for bf16, at 4x
the throughput of plain fp32 matmul.
"""
import os
import sys

sys.path.insert(0, "/opt/trn_rl_repo")

import numpy as np

import concourse.bacc as bacc
import concourse.mybir as mybir
import concourse.tile as tile
from concourse.bass_utils import run_bass_kernel_spmd

F32 = mybir.dt.float32
F32R = mybir.dt.float32r
ADD = mybir.AluOpType.add
MAX = mybir.AluOpType.max

NC = 8          # cores
L = 4           # levels
D = 1024        # in features
H = 1024        # hidden
KC = D // 128   # contraction chunks

LAST_RESULTS = None       # BassKernelResults of the most recent run (for test.py)
_PROGRAM_CACHE = {}


def _row_tiles(c):
    """Split a per-level capacity (multiple of 128, >=256) into row-tile
    sizes, all >=256 (f32r matmuls need a moving dim >=256 for full rate)."""
    tiles = [512] * (c // 512)
    rem = c % 512
    if rem == 128:
        # replace a 512 tile with 384 + 256 to keep every tile >= 256
        tiles[-1:] = [384, 256]
    elif rem:
        tiles.append(rem)
    return tiles


def _build_program(caps):
    """Build + compile the SPMD program for per-level capacities `caps`."""
    r_core = sum(caps)
    nc = bacc.Bacc("TRN2", target_bir_lowering=False, debug=False, num_devices=NC)
    xT = nc.dram_tensor("xT", [D, r_core], F32R, kind="ExternalInput")
    W1 = nc.dram_tensor("W1", [L, D, H], F32R, kind="ExternalInput")
    W2 = nc.dram_tensor("W2", [L, H, H], F32R, kind="ExternalInput")
    W3 = nc.dram_tensor("W3", [L, H, 1], F32R, kind="ExternalInput")
    b1 = nc.dram_tensor("b1", [L, H], F32, kind="ExternalInput")
    b2 = nc.dram_tensor("b2", [L, H], F32, kind="ExternalInput")
    b3 = nc.dram_tensor("b3", [L, 1], F32, kind="ExternalInput")
    out = nc.dram_tensor("out", [1, r_core], F32, kind="ExternalOutput")

    xT_r = xT.rearrange("(kc p) r -> p kc r", p=128)  # [128, KC, r_core]

    with tile.TileContext(nc) as tc:
        with (
            tc.tile_pool(name="wpool", bufs=2) as wpool,
            tc.tile_pool(name="bpool", bufs=2) as bpool,
            tc.tile_pool(name="xpool", bufs=2) as xpool,
            tc.tile_pool(name="hpool", bufs=1) as hpool,
            tc.tile_pool(name="opool", bufs=3) as opool,
            tc.tile_pool(name="ps", bufs=7, space="PSUM") as ps,
            tc.tile_pool(name="ps3", bufs=1, space="PSUM") as ps3,
        ):
            off = 0
            for lvl in range(L):
                cap = caps[lvl]
                if cap == 0:
                    continue
                # For level 0, issue the first row-tile's x DMA before the
                # weight DMAs so the PE can start as soon as the first weight
                # chunk lands instead of waiting behind 8.5MB of weights.
                pre_x = None
                if lvl == 0:
                    rt0 = _row_tiles(cap)[0]
                    pre_x = xpool.tile([128, KC, rt0], F32R, tag="x")
                    # per-kc chunk DMAs: if region-level deps are tracked the
                    # first matmul starts after just 256KB of x
                    for kc in range(KC):
                        nc.sync.dma_start(pre_x[:, kc, :], xT_r[:, kc, 0:rt0])
                # Tiny bias/W3 tiles first so evictions never wait behind the
                # 8MB of W1/W2 weight DMA.
                w3t = bpool.tile([128, KC], F32R, tag="w3")
                nc.sync.dma_start(w3t[:], W3[lvl].rearrange("(kc p) o -> p (kc o)", p=128))
                b1t = bpool.tile([128, H // 128], F32, tag="b1")
                nc.sync.dma_start(b1t[:], b1[lvl].rearrange("(mc p) -> p mc", p=128))
                b2t = bpool.tile([128, H // 128], F32, tag="b2")
                nc.sync.dma_start(b2t[:], b2[lvl].rearrange("(mc p) -> p mc", p=128))
                b3t = bpool.tile([1, 1], F32, tag="b3")
                nc.sync.dma_start(b3t[:], b3[lvl:lvl + 1, :])
                # Per-kc weight tiles so the first matmuls only wait on the
                # first 512KB of weight DMA, and level l+1 prefetch
                # double-buffers against level l (bufs=2 per tag).
                w1k = []
                w2k = []
                for kc in range(KC):
                    t1 = wpool.tile([128, H], F32R, tag=f"w1k{kc}")
                    nc.sync.dma_start(t1[:], W1[lvl][kc * 128:(kc + 1) * 128, :])
                    w1k.append(t1)
                for kc in range(KC):
                    t2 = wpool.tile([128, H], F32R, tag=f"w2k{kc}")
                    nc.sync.dma_start(t2[:], W2[lvl][kc * 128:(kc + 1) * 128, :])
                    w2k.append(t2)

                for ti, rt in enumerate(_row_tiles(cap)):
                    if pre_x is not None and ti == 0:
                        x_t = pre_x
                    else:
                        x_t = xpool.tile([128, KC, rt], F32R, tag="x")
                        nc.sync.dma_start(x_t[:], xT_r[:, :, off:off + rt])

                    # L1 runs kc-outer in two 4-bank halves: the first matmul
                    # only depends on w1k[0] + x_t, so the PE ramps with the
                    # weight DMA stream instead of waiting for all of W1.
                    h1 = hpool.tile([128, H // 128, rt], F32R, tag="h1")
                    for half in range(2):
                        mcs = range(4 * half, 4 * half + 4)
                        accs = {mc: ps.tile([128, rt], F32, tag="acc", name="acc")
                                for mc in mcs}
                        for kc in range(KC):
                            for mc in mcs:
                                nc.tensor.matmul(
                                    accs[mc][:], w1k[kc][:, mc * 128:(mc + 1) * 128],
                                    x_t[:, kc, :], start=(kc == 0), stop=(kc == KC - 1))
                        for mc in mcs:
                            nc.vector.tensor_scalar(
                                h1[:, mc, :], accs[mc][:], b1t[:, mc:mc + 1], 0.0, ADD, MAX)

                    h2 = hpool.tile([128, H // 128, rt], F32R, tag="h2")
                    for half in range(2):
                        mcs = range(4 * half, 4 * half + 4)
                        accs = {mc: ps.tile([128, rt], F32, tag="acc", name="acc")
                                for mc in mcs}
                        for kc in range(H // 128):
                            for mc in mcs:
                                nc.tensor.matmul(
                                    accs[mc][:], w2k[kc][:, mc * 128:(mc + 1) * 128],
                                    h1[:, kc, :], start=(kc == 0), stop=(kc == H // 128 - 1))
                        for mc in mcs:
                            nc.vector.tensor_scalar(
                                h2[:, mc, :], accs[mc][:], b2t[:, mc:mc + 1], 0.0, ADD, MAX)

                    acc3 = ps3.tile([1, rt], F32)
                    for kc in range(H // 128):
                        nc.tensor.matmul(acc3[:], w3t[:, kc:kc + 1], h2[:, kc, :],
                                         start=(kc == 0), stop=(kc == H // 128 - 1))
                    o_t = opool.tile([1, rt], F32, tag="o")
                    nc.vector.tensor_scalar(o_t[:], acc3[:], b3t[:], None, ADD)
                    nc.sync.dma_start(out[:, off:off + rt], o_t[:])
                    off += rt
    nc.compile()
    return nc


def kernel(x, levels, W1, b1, W2, b2, W3, b3):
    global LAST_RESULTS
    x = np.ascontiguousarray(np.asarray(x, dtype=np.float32))
    levels = np.asarray(levels)
    n = x.shape[0]

    # --- host-side routing: sort rows by level, deal evenly to cores ---
    order = np.argsort(levels, kind="stable")
    counts = np.bincount(np.asarray(levels, dtype=np.int64), minlength=L)[:L]

    # per-level capacity shared by all cores: ceil(max per-core count / 128)*128,
    # min 256 (row tiles below 256 lose f32r full rate)
    caps = []
    for lvl in range(L):
        per_core_max = -(-int(counts[lvl]) // NC)
        caps.append(max(-(-per_core_max // 128) * 128, 256) if per_core_max else 0)
    r_core = sum(caps)

    # per-core padded index lists + validity masks
    idx = np.zeros((NC, r_core), dtype=np.int64)
    valid = np.zeros((NC, r_core), dtype=bool)
    lvl_start = np.concatenate([[0], np.cumsum(counts)])
    seg_off = 0
    for lvl in range(L):
        rows = order[lvl_start[lvl]:lvl_start[lvl + 1]]
        nl = len(rows)
        q, rem = divmod(nl, NC)
        start = 0
        for c in range(NC):
            cnt = q + (1 if c < rem else 0)
            idx[c, seg_off:seg_off + cnt] = rows[start:start + cnt]
            valid[c, seg_off:seg_off + cnt] = True
            start += cnt
        seg_off += caps[lvl]

    key = tuple(caps)
    nc = _PROGRAM_CACHE.get(key)
    if nc is None:
        nc = _build_program(caps)
        _PROGRAM_CACHE[key] = nc

    in_maps = []
    for c in range(NC):
        xTc = np.ascontiguousarray(x[idx[c]].T)  # [D, r_core]
        in_maps.append({
            "xT": xTc,
            "W1": np.asarray(W1, dtype=np.float32),
            "W2": np.asarray(W2, dtype=np.float32),
            "W3": np.asarray(W3, dtype=np.float32),
            "b1": np.asarray(b1, dtype=np.float32),
            "b2": np.asarray(b2, dtype=np.float32),
            "b3": np.asarray(b3, dtype=np.float32),
        })

    trace = bool(os.environ.get("BASS_KERNEL_TRACE"))
    res = run_bass_kernel_spmd(nc, in_maps, core_ids=list(range(NC)), trace=trace)
    LAST_RESULTS = res

    result = np.zeros((n, 1), dtype=np.float32)
    for c in range(NC):
        o = np.asarray(res.results[c]["out"]).reshape(-1)
        result[idx[c][valid[c]], 0] = o[valid[c]]
    return result
